# revision 13
# baseline (speedup 1.0000x reference)
"""GAT backbone (2-layer, 2-head, N=40000, E=640000+self-loops) on 8 trn2 NeuronCores.

v2 strategy (graph/data parallel; bf16 data path):
  - Nodes sharded by contiguous range: core c owns nodes [5000c, 5000(c+1)).
  - Key algebra: out[i,h] = W_h (sum_j alpha^h_ij x_j), so the per-head
    projection is applied AFTER aggregation. Only x (128 cols) is gathered
    per edge, not per-head h (256 cols).
  - Per layer: per-node logits [es0,es1,ed0,ed1] = x @ (W^T A) via PE; node
    row table mine[n] = [x|1|es0|es1] in bf16 (256 cols, 512B rows),
    AllGather -> replicated fullr.
  - Edges pre-partitioned by dst-owner core, sorted by dst, grouped into
    128-dst chunks, bucketed by src half (int16 gather-index limit), padded
    to 128-multiples uniform across cores. Per super-chunk: dma_gather of
    512B rows by src; dma_gather of 256B [ed0,ed1,...] rows by local dst.
  - Per-edge p0 = exp(lrelu(es0+ed0)) and r = exp(lrelu1-lrelu0) computed
    compactly (2 floats/edge); DVE builds per-block selectors
    sp0[e,d] = (iota==dstrel)*p0 (bf16, 2x mode) and sp1 = sp0*r = mask*p1.
  - Tensor engine: psum[d, 0:129] += sp_h^T @ [x|1] accumulates weighted
    x-sum and softmax denominator per dst, per head (bf16 operands).
  - Finalize per 128-dst chunk: q_h = U_h/s_h (bf16) -> transpose ->
    z = q0 @ W0^T + q1 @ W1^T -> elu(0.5 z + bias) -> out (fp32).

kernel(**inputs) takes FULL inputs, returns (x, h1, h2) like the reference.
"""

import sys

import numpy as np

_TRN_REPO = "/opt/trn_rl_repo"
if _TRN_REPO not in sys.path:
    sys.path.insert(0, _TRN_REPO)

# ---------------------------------------------------------------- constants
NCORES = 8
NT = 40000          # total nodes
NPC = NT // NCORES  # nodes per core (5000)
D = 128             # input dim
H = 2               # heads
C = 128             # per-head channels
OC = H * C          # 256
L = 2
NEG = 0.2
ROWW = 256          # bf16 row: [x(128) | 1 | es0 | es1 | zeros...] = 512B
EDW = 64            # ed-row gather width in f32 (256B min elem)
SC = 2              # dst-chunks per super-chunk (gather batching)
SPLITA = 2560       # local rows per core in gather table A (rest in B)


# ---------------------------------------------------------------- host prep
def _wrap(flat: np.ndarray) -> np.ndarray:
    """[n] -> [128, n/16]: element i at [i%16, i//16], replicated 8x down."""
    n = flat.shape[0]
    assert n % 16 == 0
    blk = flat.reshape(n // 16, 16).T  # [16, n/16]
    return np.tile(blk, (8, 1))


def plan_edges(edge_index: np.ndarray, nt: int, ncores: int):
    """Partition by dst owner, sort by dst, chunk by 128 dsts, bucket by src
    half, pad each (chunk, half) to a 128 multiple (uniform across cores).

    Returns per-core arrays + compile-time block counts BL[k], BH[k].
    """
    npc = nt // ncores
    spb = npc - SPLITA
    nnch = (npc + 127) // 128
    loops = np.arange(nt, dtype=np.int64)
    src = np.concatenate([edge_index[0].astype(np.int64), loops])
    dst = np.concatenate([edge_index[1].astype(np.int64), loops])

    # per (core, chunk): lo/hi edge lists (src-table-row, dstl).
    # lo = srcs whose local row < SPLITA (gather table A: AllGather of each
    # core's first SPLITA rows); hi = rest (table B).
    per = [[([], []) for _ in range(nnch)] for _ in range(ncores)]
    order = np.argsort(dst, kind="stable")
    src = src[order]
    dst = dst[order]
    core_of = dst // npc
    s_core = src // npc
    s_loc = src - s_core * npc
    in_a = s_loc < SPLITA
    rowa = s_core * SPLITA + s_loc
    rowb = s_core * spb + (s_loc - SPLITA)
    for c in range(ncores):
        sel = core_of == c
        a_c = in_a[sel]
        ra_c = rowa[sel]
        rb_c = rowb[sel]
        dl_c = dst[sel] - npc * c
        ch_c = dl_c // 128
        for k in range(nnch):
            m = ch_c == k
            a_k = a_c[m]
            ra_k = ra_c[m]
            rb_k = rb_c[m]
            d_k = dl_c[m]
            per[c][k] = ((ra_k[a_k], d_k[a_k]), (rb_k[~a_k], d_k[~a_k]))

    BL = [0] * nnch
    BH = [0] * nnch
    for k in range(nnch):
        BL[k] = max(1, -(-max(len(per[c][k][0][0]) for c in range(ncores)) // 128))
        BH[k] = max(1, -(-max(len(per[c][k][1][0]) for c in range(ncores)) // 128))

    scs = [list(range(s, min(s + SC, nnch))) for s in range(0, nnch, SC)]

    plans = []
    for c in range(ncores):
        gxlo_parts, gxhi_parts, sixd_parts, dstrel_parts = [], [], [], []
        for ks in scs:
            lo_g, lo_d, hi_g, hi_d = [], [], [], []
            for k in ks:
                (ls, ld), (hs, hd) = per[c][k]
                pl = 128 * BL[k] - len(ls)
                ph = 128 * BH[k] - len(hs)
                lo_g.append(np.concatenate([ls, np.zeros(pl, np.int64)]))
                lo_d.append(np.concatenate([ld, np.full(pl, -1, np.int64)]))
                hi_g.append(np.concatenate([hs, np.zeros(ph, np.int64)]))
                hi_d.append(np.concatenate([hd, np.full(ph, -1, np.int64)]))
            lo_g = np.concatenate(lo_g)
            hi_g = np.concatenate(hi_g)
            lo_d = np.concatenate(lo_d)
            hi_d = np.concatenate(hi_d)
            full_d = np.concatenate([lo_d, hi_d])  # dstl, pads -1
            gxlo_parts.append(_wrap(lo_g.astype(np.int16)))
            gxhi_parts.append(_wrap(hi_g.astype(np.int16)))
            sixd_parts.append(_wrap(np.where(full_d < 0, npc, full_d)
                                    .astype(np.int16)))
            # dstrel per (partition, block): edge i -> (i%128, i//128)
            nbl = full_d.shape[0] // 128
            rel = np.full((128, nbl), -1.0, np.float32)
            # chunk of each block in stream order [lo ks..., hi ks...]
            kof = []
            for k in ks:
                kof += [k] * BL[k]
            for k in ks:
                kof += [k] * BH[k]
            for i, dv in enumerate(full_d):
                if dv >= 0:
                    rel[i % 128, i // 128] = dv - 128 * kof[i // 128]
            dstrel_parts.append(rel)
        plans.append({
            "gxlo": np.concatenate(gxlo_parts, axis=1),
            "gxhi": np.concatenate(gxhi_parts, axis=1),
            "sixd": np.concatenate(sixd_parts, axis=1),
            "dstrel": np.concatenate(dstrel_parts, axis=1),
        })
    return plans, BL, BH, scs


def build_vmat(W: np.ndarray, att_src: np.ndarray, att_dst: np.ndarray):
    """V [L, D, 4]: x @ V = [es0, es1, ed0, ed1] per node."""
    V = np.zeros((L, D, 4), np.float32)
    for layer in range(L):
        A = np.zeros((OC, 4), np.float32)
        A[0:C, 0] = att_src[layer, 0]
        A[C:OC, 1] = att_src[layer, 1]
        A[0:C, 2] = att_dst[layer, 0]
        A[C:OC, 3] = att_dst[layer, 1]
        V[layer] = W[layer].T.astype(np.float32) @ A
    return V


# ------------------------------------------------------------ bass program
def build_program(nt, ncores, BL, BH, scs, upto="full", repeat=1):
    from contextlib import ExitStack

    import concourse.bacc as bacc
    import concourse.tile as tile
    from concourse import mybir

    npc = nt // ncores
    ntA = ncores * SPLITA
    ntB = nt - ntA
    cha = SPLITA // 128
    assert SPLITA % 128 == 0 and ntA < 32768 and ntB < 32768
    nnch = (npc + 127) // 128
    f32 = mybir.dt.float32
    bf16 = mybir.dt.bfloat16
    i16 = mybir.dt.int16
    WLO = sum(BL) * 8      # gxlo idx cols
    WHI = sum(BH) * 8
    CBT = sum(BL) + sum(BH)  # total blocks
    WSD = CBT * 8
    eq = mybir.AluOpType.is_equal
    mult = mybir.AluOpType.mult
    add = mybir.AluOpType.add
    sub = mybir.AluOpType.subtract
    AF = mybir.ActivationFunctionType

    nc = bacc.Bacc("TRN2", target_bir_lowering=False, debug=False,
                   num_devices=ncores, num_swdge_queues=4)

    x0 = nc.dram_tensor("x0", [npc, D], f32, kind="ExternalInput")
    WTm = nc.dram_tensor("WTm", [L, D, OC], bf16, kind="ExternalInput")
    Vm = nc.dram_tensor("Vm", [L, D, 4], bf16, kind="ExternalInput")
    bv = nc.dram_tensor("bv", [L, D], f32, kind="ExternalInput")
    gxlo = nc.dram_tensor("gxlo", [128, WLO], i16, kind="ExternalInput")
    gxhi = nc.dram_tensor("gxhi", [128, WHI], i16, kind="ExternalInput")
    sixd = nc.dram_tensor("sixd", [128, WSD], i16, kind="ExternalInput")
    dstr = nc.dram_tensor("dstr", [128, CBT], f32, kind="ExternalInput")
    out1 = nc.dram_tensor("out1", [npc, D], f32, kind="ExternalOutput")
    out2 = nc.dram_tensor("out2", [npc, D], f32, kind="ExternalOutput")

    mine = nc.dram_tensor("mine", [npc, ROWW], bf16)
    fullrA = nc.dram_tensor("fullrA", [ntA + 32, ROWW], bf16,
                            addr_space="Shared")
    fullrB = nc.dram_tensor("fullrB", [ntB + 32, ROWW], bf16,
                            addr_space="Shared")
    edloc = nc.dram_tensor("edloc", [npc + 16, EDW], f32)

    groups = [list(range(ncores))]
    CBMAX = max(sum(BL[k] for k in ks) + sum(BH[k] for k in ks) for ks in scs)
    nfull = npc // 128
    rem = npc - 128 * nfull

    with tile.TileContext(nc) as tc, ExitStack() as ctx:
        cpool = ctx.enter_context(tc.tile_pool(name="const", bufs=1))
        wpool = ctx.enter_context(tc.tile_pool(name="wts", bufs=1))
        npool = ctx.enter_context(tc.tile_pool(name="nodes", bufs=2))
        gpool = ctx.enter_context(tc.tile_pool(name="gath", bufs=2))
        dpool = ctx.enter_context(tc.tile_pool(name="edg", bufs=2))
        ppool = ctx.enter_context(tc.tile_pool(name="pvals", bufs=2))
        spool = ctx.enter_context(tc.tile_pool(name="sprime", bufs=3))
        fpool = ctx.enter_context(tc.tile_pool(name="final", bufs=2))
        psn = ctx.enter_context(tc.tile_pool(name="psn", bufs=2, space="PSUM"))
        pss = ctx.enter_context(tc.tile_pool(name="pss", bufs=1, space="PSUM"))

        # constants
        ident = cpool.tile([128, 128], bf16)
        nc.vector.memset(ident[:], 1.0)
        nc.gpsimd.affine_select(ident[:], ident[:], pattern=[[1, 128]], base=0,
                                channel_multiplier=-1, compare_op=eq, fill=0.0)
        iota_bf = cpool.tile([128, 128], bf16)
        nc.gpsimd.iota(iota_bf[:], pattern=[[1, 128]], base=0,
                       channel_multiplier=0,
                       allow_small_or_imprecise_dtypes=True)
        ones_row = cpool.tile([1, 128], f32)
        nc.vector.memset(ones_row[:], 1.0)
        zero_t = cpool.tile([16, EDW], f32)
        nc.vector.memset(zero_t[:], 0.0)

        # index tables (persist across both layers)
        gxlo_sb = cpool.tile([128, WLO], i16)
        gxhi_sb = cpool.tile([128, WHI], i16)
        sixd_sb = cpool.tile([128, WSD], i16)
        dstr_sb = cpool.tile([128, CBT], f32)
        nc.sync.dma_start(gxlo_sb[:], gxlo[:])
        nc.sync.dma_start(gxhi_sb[:], gxhi[:])
        nc.sync.dma_start(sixd_sb[:], sixd[:])
        nc.sync.dma_start(dstr_sb[:], dstr[:])

        # node features chunked in SBUF: [128, nnch, 128] f32
        xall = wpool.tile([128, nnch, D], f32, tag="xall")
        nc.sync.dma_start(
            xall[:, 0:nfull, :],
            x0[0:128 * nfull, :].rearrange("(n p) f -> p n f", p=128))
        if rem:
            nc.sync.dma_start(xall[:rem, nfull, :], x0[128 * nfull:npc, :])

        for layer in [ly for _ in range(repeat) for ly in range(L)]:
            outl = out1 if layer == 0 else out2

            # ---- weight prep ----
            wt_sb = wpool.tile([128, OC], bf16, tag="wt_sb")
            v_sb = wpool.tile([128, 4], bf16, tag="v_sb")
            nc.sync.dma_start(wt_sb[:], WTm[layer, :, :])
            nc.sync.dma_start(v_sb[:], Vm[layer, :, :])

            brow = wpool.tile([1, 128], f32, tag="brow")
            nc.sync.dma_start(brow[:], bv[layer:layer + 1, :])
            bps = psn.tile([128, 128], f32, tag="mm32")
            nc.tensor.matmul(bps[:], ones_row[:], brow[:], start=True,
                             stop=True)
            bias_bc = wpool.tile([128, 128], f32, tag="bias_bc")
            nc.vector.tensor_copy(bias_bc[:], bps[:])

            # ---- projection: rows [x|1|es0|es1] + local ed table ----
            rt = wpool.tile([128, nnch, ROWW], bf16, tag="rt")
            edall = wpool.tile([128, nnch, EDW], f32, tag="edall")
            nc.vector.memset(rt[:, :, 129:ROWW], 0.0)
            nc.vector.memset(rt[:, :, 128:129], 1.0)
            nc.vector.memset(edall[:], 0.0)
            for chk in range(nnch):
                p = min(128, npc - 128 * chk)
                nc.vector.tensor_copy(rt[:p, chk, 0:128], xall[:p, chk, :])
                xT_ps = psn.tile([128, 128], bf16, tag="tpbf")
                nc.tensor.transpose(xT_ps[:, :p], rt[:p, chk, 0:128],
                                    ident[:p, :p])
                xT = npool.tile([128, 128], bf16, tag="xTs")
                nc.vector.tensor_copy(xT[:, :p], xT_ps[:, :p])
                es_ps = psn.tile([128, 4], f32, tag="mm32")
                nc.tensor.matmul(es_ps[:p, :], xT[:, :p], v_sb[:], start=True,
                                 stop=True)
                nc.vector.tensor_copy(rt[:p, chk, 129:131], es_ps[:p, 0:2])
                nc.vector.tensor_copy(edall[:p, chk, 0:2], es_ps[:p, 2:4])
                if chk == cha - 1:
                    # A-half rows done: ship + start first collective early
                    nc.sync.dma_start(
                        mine[0:SPLITA, :].rearrange("(n p) f -> p n f",
                                                    p=128),
                        rt[:, 0:cha, :])
                    if upto != "node":
                        if ncores == 1:
                            nc.sync.dma_start(fullrA[0:SPLITA, :],
                                              mine[0:SPLITA, :])
                        else:
                            nc.gpsimd.collective_compute(
                                "AllGather", mybir.AluOpType.bypass,
                                ins=[mine[0:SPLITA, :]],
                                outs=[fullrA[0:ntA, :]],
                                replica_groups=groups)
            if nfull > cha:
                nc.sync.dma_start(
                    mine[SPLITA:128 * nfull, :].rearrange("(n p) f -> p n f",
                                                          p=128),
                    rt[:, cha:nfull, :])
            if rem:
                nc.sync.dma_start(mine[128 * nfull:npc, :],
                                  rt[:rem, nfull, :])
            if nfull:
                nc.sync.dma_start(
                    edloc[0:128 * nfull, :].rearrange("(n p) f -> p n f",
                                                      p=128),
                    edall[:, 0:nfull, :])
            if rem:
                nc.sync.dma_start(edloc[128 * nfull:npc, :],
                                  edall[:rem, nfull, :])
            nc.sync.dma_start(edloc[npc:npc + 16, :], zero_t[:])

            # ---- second AllGather (B half) ----
            if upto != "node":
                if ncores == 1:
                    nc.sync.dma_start(fullrB[0:npc - SPLITA, :],
                                      mine[SPLITA:npc, :])
                else:
                    nc.gpsimd.collective_compute(
                        "AllGather", mybir.AluOpType.bypass,
                        ins=[mine[SPLITA:npc, :]], outs=[fullrB[0:ntB, :]],
                        replica_groups=groups)

            if upto in ("node", "collective"):
                zz = wpool.tile([128, 128], f32, tag="zz")
                nc.vector.memset(zz[:], 0.0)
                for chk in range(nnch):
                    p = min(128, npc - 128 * chk)
                    nc.sync.dma_start(outl[128 * chk:128 * chk + p, :],
                                      zz[:p, :])
                continue
            gather_only = upto in ("gatherG", "gatherD", "spmm")

            # ---- super-chunk loop ----
            co_l = co_h = co_s = co_b = 0
            qctr = 0
            for ks in scs:
                nbl = sum(BL[k] for k in ks)
                nbh = sum(BH[k] for k in ks)
                ncb = nbl + nbh
                G = gpool.tile([128, CBMAX, ROWW], bf16, tag="G")
                nc.gpsimd.dma_gather(
                    G[:, 0:nbl, :], fullrA[0:ntA, :],
                    gxlo_sb[:, co_l:co_l + nbl * 8], 128 * nbl, 128 * nbl,
                    ROWW, single_packet=False, queue_num=qctr % 4)
                qctr += 1
                nc.gpsimd.dma_gather(
                    G[:, nbl:ncb, :], fullrB[0:ntB, :],
                    gxhi_sb[:, co_h:co_h + nbh * 8], 128 * nbh, 128 * nbh,
                    ROWW, single_packet=False, queue_num=qctr % 4)
                qctr += 1
                if upto == "gatherG":
                    for i, k in enumerate(ks):
                        p = min(128, npc - 128 * k)
                        zz2 = fpool.tile([128, 128], f32, tag="zz2")
                        nc.vector.tensor_copy(zz2[:], G[:, i, 0:128])
                        nc.sync.dma_start(outl[128 * k:128 * k + p, :],
                                          zz2[:p, :])
                    co_l += nbl * 8
                    co_h += nbh * 8
                    co_s += ncb * 8
                    co_b += ncb
                    continue
                Dt = dpool.tile([128, CBMAX, EDW], f32, tag="Dt")
                nc.gpsimd.dma_gather(
                    Dt[:, 0:ncb, :], edloc[:],
                    sixd_sb[:, co_s:co_s + ncb * 8], 128 * ncb, 128 * ncb,
                    EDW, single_packet=False, queue_num=qctr % 4)
                qctr += 1

                # per-edge compact: T = es + ed ; lrelu ; p0 ; r = exp(d1-d0)
                esf = ppool.tile([128, CBMAX, 2], f32, tag="esf")
                nc.vector.tensor_copy(esf[:, 0:ncb, :], G[:, 0:ncb, 129:131])
                T = ppool.tile([128, CBMAX, 2], f32, tag="T")
                nc.vector.tensor_tensor(T[:, 0:ncb, :], esf[:, 0:ncb, :],
                                        Dt[:, 0:ncb, 0:2], add)
                TL = ppool.tile([128, CBMAX, 2], f32, tag="TL")
                nc.scalar.activation(TL[:, 0:ncb, :], T[:, 0:ncb, :],
                                     AF.Prelu, alpha=NEG)
                RD = ppool.tile([128, CBMAX], f32, tag="RD")
                nc.vector.tensor_tensor(RD[:, 0:ncb], TL[:, 0:ncb, 1],
                                        TL[:, 0:ncb, 0], sub)
                P0 = ppool.tile([128, CBMAX], f32, tag="P0")
                nc.scalar.activation(P0[:, 0:ncb], TL[:, 0:ncb, 0], AF.Exp)
                R = ppool.tile([128, CBMAX], f32, tag="R")
                nc.scalar.activation(R[:, 0:ncb], RD[:, 0:ncb], AF.Exp)
                if upto == "gatherD":
                    for i, k in enumerate(ks):
                        p = min(128, npc - 128 * k)
                        zz2 = fpool.tile([128, 128], f32, tag="zz2")
                        nc.vector.tensor_scalar_mul(zz2[:], G[:, i, 0:128],
                                                    P0[:, i:i + 1])
                        nc.sync.dma_start(outl[128 * k:128 * k + p, :],
                                          zz2[:p, :])
                    co_l += nbl * 8
                    co_h += nbh * 8
                    co_s += ncb * 8
                    co_b += ncb
                    continue

                # block order in G: [lo(k) for k in ks] + [hi(k) for k in ks]
                kof = []
                for k in ks:
                    kof += [k] * BL[k]
                for k in ks:
                    kof += [k] * BH[k]
                first = {}
                last = {}
                for b, k in enumerate(kof):
                    last[k] = b
                for b in range(len(kof) - 1, -1, -1):
                    first[kof[b]] = b
                psums = {}
                for i, k in enumerate(ks):
                    psums[k] = [
                        pss.tile([128, 132], f32, tag=f"sg{i}{h}",
                                 name=f"seg_l{layer}_k{k}_h{h}")
                        for h in range(H)]
                for b, k in enumerate(kof):
                    sp0 = spool.tile([128, 128], bf16, tag="sp0")
                    nc.vector.tensor_scalar(
                        sp0[:], iota_bf[:],
                        dstr_sb[:, co_b + b:co_b + b + 1],
                        P0[:, b:b + 1], eq, mult)
                    sp1 = spool.tile([128, 128], bf16, tag="sp1")
                    if b % 5 < 3:
                        nc.scalar.activation(sp1[:], sp0[:], AF.Copy,
                                             scale=R[:, b:b + 1])
                    else:
                        nc.vector.tensor_scalar_mul(sp1[:], sp0[:],
                                                    R[:, b:b + 1])
                    st = b == first[k]
                    en = b == last[k]
                    nc.tensor.matmul(psums[k][0][:, 0:129], sp0[:],
                                     G[:, b, 0:129], start=st, stop=en)
                    nc.tensor.matmul(psums[k][1][:, 0:129], sp1[:],
                                     G[:, b, 0:129], start=st, stop=en)

                if upto == "spmm":
                    for i, k in enumerate(ks):
                        p = min(128, npc - 128 * k)
                        zz2 = fpool.tile([128, 132], f32, tag="zz2")
                        nc.vector.tensor_copy(zz2[:, 0:129],
                                              psums[k][0][:, 0:129])
                        nc.vector.tensor_copy(zz2[:, 0:129],
                                              psums[k][1][:, 0:129])
                        nc.sync.dma_start(outl[128 * k:128 * k + p, :],
                                          zz2[:p, :])
                    co_l += nbl * 8
                    co_h += nbh * 8
                    co_s += ncb * 8
                    co_b += ncb
                    continue

                # final per dst-chunk
                for i, k in enumerate(ks):
                    p = min(128, npc - 128 * k)
                    r0 = fpool.tile([128, 1], f32, tag="r0")
                    r1 = fpool.tile([128, 1], f32, tag="r1")
                    s0 = fpool.tile([128, 1], f32, tag="s0")
                    s1 = fpool.tile([128, 1], f32, tag="s1")
                    nc.vector.tensor_copy(s0[:p, :], psums[k][0][:p, 128:129])
                    nc.vector.tensor_copy(s1[:p, :], psums[k][1][:p, 128:129])
                    nc.vector.reciprocal(r0[:p, :], s0[:p, :])
                    nc.vector.reciprocal(r1[:p, :], s1[:p, :])
                    q0 = fpool.tile([128, 128], bf16, tag="q0")
                    q1 = fpool.tile([128, 128], bf16, tag="q1")
                    nc.vector.tensor_scalar_mul(q0[:p, :],
                                                psums[k][0][:p, 0:128],
                                                r0[:p, :])
                    nc.scalar.activation(q1[:p, :], psums[k][1][:p, 0:128],
                                          AF.Copy, scale=r1[:p, :])
                    qt_ps0 = psn.tile([128, 128], bf16, tag="tpbf")
                    nc.tensor.transpose(qt_ps0[:, :p], q0[:p, :],
                                        ident[:p, :p])
                    qT0 = fpool.tile([128, 128], bf16, tag="qT0")
                    nc.vector.tensor_copy(qT0[:, :p], qt_ps0[:, :p])
                    qt_ps1 = psn.tile([128, 128], bf16, tag="tpbf")
                    nc.tensor.transpose(qt_ps1[:, :p], q1[:p, :],
                                        ident[:p, :p])
                    qT1 = fpool.tile([128, 128], bf16, tag="qT1")
                    nc.scalar.activation(qT1[:, :p], qt_ps1[:, :p], AF.Copy)
                    zps = psn.tile([128, 128], f32, tag="mm32")
                    nc.tensor.matmul(zps[:p, :], qT0[:, :p], wt_sb[:, 0:128],
                                     start=True, stop=False)
                    nc.tensor.matmul(zps[:p, :], qT1[:, :p], wt_sb[:, 128:OC],
                                     start=False, stop=True)
                    z1 = fpool.tile([128, 128], f32, tag="z1")
                    nc.vector.tensor_scalar_mul(z1[:p, :], zps[:p, :], 0.5)
                    nc.vector.tensor_tensor(z1[:p, :], z1[:p, :],
                                            bias_bc[:p, :], add)
                    rl = fpool.tile([128, 128], f32, tag="rl")
                    nc.scalar.activation(rl[:p, :], z1[:p, :], AF.Relu)
                    nc.vector.tensor_scalar_min(z1[:p, :], z1[:p, :], 0.0)
                    ex = fpool.tile([128, 128], f32, tag="ex")
                    nc.scalar.activation(ex[:p, :], z1[:p, :], AF.Exp)
                    nc.vector.tensor_tensor(rl[:p, :], rl[:p, :], ex[:p, :],
                                            add)
                    nc.vector.tensor_scalar_add(xall[:p, k, :], rl[:p, :],
                                                -1.0)

                co_l += nbl * 8
                co_h += nbh * 8
                co_s += ncb * 8
                co_b += ncb

            # write layer output from xall
            nc.sync.dma_start(
                outl[0:128 * nfull, :].rearrange("(n p) f -> p n f", p=128),
                xall[:, 0:nfull, :])
            if rem:
                nc.sync.dma_start(outl[128 * nfull:npc, :],
                                  xall[:rem, nfull, :])

    nc.compile()
    return nc


# ---------------------------------------------------------------- kernel()
def make_in_maps(x, edge_index, W, att_src, att_dst, bias):
    import ml_dtypes

    x = np.asarray(x, np.float32)
    edge_index = np.asarray(edge_index)
    W = np.asarray(W, np.float32)
    att_src = np.asarray(att_src, np.float32)
    att_dst = np.asarray(att_dst, np.float32)
    bias = np.asarray(bias, np.float32)

    plans, BL, BH, scs = plan_edges(edge_index, NT, NCORES)
    V = build_vmat(W, att_src, att_dst)
    WT = np.ascontiguousarray(W.transpose(0, 2, 1))  # [L, D, OC]

    in_maps = []
    for c in range(NCORES):
        in_maps.append({
            "x0": x[NPC * c:NPC * (c + 1)].copy(),
            "WTm": WT.astype(ml_dtypes.bfloat16),
            "Vm": V.astype(ml_dtypes.bfloat16),
            "bv": bias,
            "gxlo": plans[c]["gxlo"], "gxhi": plans[c]["gxhi"],
            "sixd": plans[c]["sixd"], "dstr": plans[c]["dstrel"],
        })
    return in_maps, BL, BH, scs


def _run(x, edge_index, W, att_src, att_dst, bias, trace=False):
    from concourse import bass_utils

    in_maps, BL, BH, scs = make_in_maps(x, edge_index, W, att_src, att_dst,
                                        bias)
    nc = build_program(NT, NCORES, BL, BH, scs)
    res = bass_utils.run_bass_kernel_spmd(
        nc, in_maps, list(range(NCORES)), trace=trace)
    x1 = np.concatenate([res.results[c]["out1"] for c in range(NCORES)],
                        axis=0)
    x2 = np.concatenate([res.results[c]["out2"] for c in range(NCORES)],
                        axis=0)
    x = np.asarray(x, np.float32)
    return (x, x1, x2), res


def kernel(x, edge_index, W, att_src, att_dst, bias):
    out, _ = _run(x, edge_index, W, att_src, att_dst, bias, trace=False)
    return out
